# revision 1
# baseline (speedup 1.0000x reference)
"""TRN2 Bass kernel for nn_AttentionStoreProcessor (dense transformer attention).

Full (unsharded) inputs in, full output out. Internally:
  - CAPE rotation + softmax scale folded into Wq/Wk on host (exact linear algebra,
    per-frame 4x4 block-diagonal right-multiply).
  - Heads padded 20 -> 24 and tensor-parallel sharded 3 heads/core across 8 cores
    (zero weights for pad heads; their output contribution is exactly zero).
  - Per core: hs^T via PE transposes; fused QKV projections (float32r ~= tf32
    precision at full PE rate); scores^T per (head, kt-tile); max-free softmax
    (scores are O(10), exp is safe in fp32) with sums obtained via a ones-column
    appended to V in the PV matmul; per-query normalization via a K=1 broadcast
    matmul; output projection from outT, overlapped per query-half; residual,
    bias and the cross-core partial-sum reduction happen on host.
"""
import numpy as np
from contextlib import ExitStack

import concourse.bacc as bacc
import concourse.mybir as mybir
import concourse.tile as tile
from concourse.bass_utils import run_bass_kernel_spmd

F32 = mybir.dt.float32
F32R = mybir.dt.float32r
AF = mybir.ActivationFunctionType

HEADS = 20
PAD_HEADS = 24
HPC = 3  # heads per core
N_CORES = 8
S = 2048  # tokens
D = 1280  # channels
HD = 64  # head dim
L = 1024  # tokens per frame
KT = D // 128  # 10 contraction tiles for projections
TOKT = S // 128  # 16 token tiles

# wpack free-dim layout (per partition):
#   [0:7680)      six 1280-wide wg blocks, order (t0g0,t0g1,t0g2,t1g0,t1g1,t1g2)
#   [7680:10240)  wv, KT tiles of 256 cols ([v_h0|v_h1|v_h2|zeros(64)])
#   [10240:10368) identity 128x128
#   [10368:10432) ones 128x64
WV_OFF = 7680
ID_OFF = 10240
ONES_OFF = 10368
WPACK_W = 10432

_CACHED_NC = None


def _build_nc():
    nc = bacc.Bacc("TRN2", debug=False, num_devices=N_CORES)

    hs = nc.dram_tensor("hs", [S, D], F32R, kind="ExternalInput").ap()
    wpack = nc.dram_tensor("wpack", [128, WPACK_W], F32R, kind="ExternalInput").ap()
    wopack = nc.dram_tensor("wopack", [128, 2560], F32R, kind="ExternalInput").ap()
    out = nc.dram_tensor("out", [S, D], F32, kind="ExternalOutput").ap()

    hs_r = hs.rearrange("(n p) d -> n p d", p=128)
    out_r = out.rearrange("(n p) d -> n p d", p=128)

    with (
        tile.TileContext(nc) as tc,
        ExitStack() as ctx,
        nc.allow_low_precision(reason="float32r (~tf32) used deliberately"),
    ):
        persist = ctx.enter_context(tc.tile_pool(name="persist", bufs=1))
        hsin_pool = tc.alloc_tile_pool(name="hsin", bufs=7)
        psT = tc.alloc_tile_pool(name="psT", bufs=8, space="PSUM")
        s1 = tc.alloc_tile_pool(name="s1", bufs=1)

        # identity + ones first (small DMA on the ACT ring so transposes can
        # start as soon as the first hs tile lands on the SP ring)
        io_sb = s1.tile([128, 192], F32R, tag="identones")
        nc.scalar.dma_start(io_sb[:], wpack[:, ID_OFF:WPACK_W])
        ident_sb = io_sb[:, 0:128]
        ones_sb = persist.tile([128, 64], F32R, tag="ones")
        nc.vector.tensor_copy(ones_sb[:], io_sb[:, 128:192])

        # hs tiles: SP ring, emitted before the big weight DMA
        hs_sb = []
        for n in range(TOKT):
            t_in = hsin_pool.tile([128, D], F32R, tag="hsin", name=f"hsin{n}")
            eng = nc.sync if n % 2 == 0 else nc.scalar
            eng.dma_start(t_in[:], hs_r[n])
            hs_sb.append(t_in)

        # projection weights (single big DMA, lands while transposes run)
        wp = s1.tile([128, ID_OFF], F32R, tag="wpack")
        nc.sync.dma_start(wp[:], wpack[:, 0:ID_OFF])
        wg_sb = [
            [wp[:, (t * 3 + g) * 1280 : (t * 3 + g + 1) * 1280] for g in range(3)]
            for t in range(2)
        ]
        wv_sb = wp[:, WV_OFF:ID_OFF]

        hsT = [s1.tile([128, S], F32R, tag=f"hsT{k}", name=f"hsT{k}") for k in range(KT)]
        QA = persist.tile([128, S], F32R, tag="QA")  # rows 0:64 qT_h0, 64:128 qT_h1
        KA = persist.tile([128, S], F32R, tag="KA")  # rows 0:64 kT_h0, 64:128 kT_h1
        QK2 = persist.tile([128, S], F32R, tag="QK2")  # rows 0:64 q2, 64:128 k2
        QB2 = persist.tile([128, S], F32R, tag="QB2")  # rows 64:128 <- q2 (shifted)
        v195 = persist.tile([128, TOKT, 195], F32R, tag="v195")

        # ones columns of v_ext (col 65h+64 = 1.0)
        for h in range(HPC):
            nc.vector.tensor_copy(v195[:, :, 65 * h + 64], ones_sb[:, 0:TOKT])

        # ---- stage T: PE-transpose hs into hsT (psum evacuation on ScalarE,
        # which is otherwise idle until the attention exps start) ----
        for grp in range(4):  # groups of 4 token tiles
            for k in range(KT):
                tp = psT.tile([128, 512], F32R, tag="ps512", name=f"tp{grp}_{k}")
                for j in range(4):
                    n = grp * 4 + j
                    nc.tensor.transpose(
                        tp[:, j * 128 : (j + 1) * 128],
                        hs_sb[n][:, k * 128 : (k + 1) * 128],
                        ident_sb,
                    )
                nc.scalar.copy(hsT[k][:, grp * 512 : (grp + 1) * 512], tp[:])

        # ---- stage P: projections ----
        # q/k groups: per 512-token chunk (4 chunks; chunk//2 selects CAPE frame t)
        for ch in range(4):
            t = ch // 2
            qs = slice(ch * 512, (ch + 1) * 512)
            for g, dest in enumerate((QA, KA, QK2)):
                pp = psT.tile([128, 512], F32, tag="ps512", name=f"pp{ch}_{g}")
                for k in range(KT):
                    nc.tensor.matmul(
                        pp[:],
                        wg_sb[t][g][:, k * 128 : (k + 1) * 128],
                        hsT[k][:, qs],
                        start=(k == 0),
                        stop=(k == KT - 1),
                    )
                nc.vector.tensor_copy(dest[:, qs], pp[:])
            # v for the 4 token tiles of this chunk (256-wide output keeps the
            # f32r matmul at 1 cyc/row; cols 192:256 are zero padding)
            for j in range(4):
                n = ch * 4 + j
                vp = psT.tile([128, 256], F32, tag="ps512", name=f"vp{n}")
                for k in range(KT):
                    nc.tensor.matmul(
                        vp[:],
                        hsT[k][:, n * 128 : (n + 1) * 128],
                        wv_sb[:, k * 256 : (k + 1) * 256],
                        start=(k == 0),
                        stop=(k == KT - 1),
                    )
                for h in range(HPC):
                    nc.vector.tensor_copy(
                        v195[:, n, 65 * h : 65 * h + 64],
                        vp[:, h * 64 : (h + 1) * 64],
                    )

        # shift q2 (QK2 rows 0:64) up to rows 64:128 so h2 scores run at base 64
        nc.sync.dma_start(QB2[64:128, :], QK2[0:64, :])

        # free stage-1 SBUF (hsT, projection weights, hs input staging)
        s1.release()
        psT.release()
        hsin_pool.release()

        # late-stage tensors (created after hsT frees up SBUF)
        persistB = ctx.enter_context(tc.tile_pool(name="persistB", bufs=1))
        u_pool = tc.alloc_tile_pool(name="u", bufs=6)
        rc_pool = tc.alloc_tile_pool(name="rc", bufs=3)
        osb_pool = tc.alloc_tile_pool(name="osb", bufs=6)
        outT01 = persistB.tile([128, S], F32R, tag="outT01")
        outT2 = persistB.tile([64, S], F32R, tag="outT2")
        oT1tmp = persistB.tile([64, S], F32R, tag="oT1tmp")
        wop = persistB.tile([128, 2560], F32R, tag="wop")
        nc.scalar.dma_start(wop[:], wopack[:])
        wo01_sb = wop[:, 0:1280]
        wo2_sb = wop[0:64, 1280:2560]

        sc_pool = tc.alloc_tile_pool(name="sc", bufs=2, space="PSUM")
        pv_pool = tc.alloc_tile_pool(name="pv", bufs=4, space="PSUM")

        def head_ops(h):
            # (kT source, rows, qT source, rows) — both at the same base
            if h == 0:
                return KA, slice(0, 64), QA, slice(0, 64)
            if h == 1:
                return KA, slice(64, 128), QA, slice(64, 128)
            return QK2, slice(64, 128), QB2, slice(64, 128)

        def score_pv(h, qh, kt, pv_tiles, name):
            ksrc, krows, qsrc, qrows = head_ops(h)
            sc = sc_pool.tile([128, 1024], F32, tag="sc", name=f"sc{name}")
            for half in range(2):
                nc.tensor.matmul(
                    sc[:, half * 512 : (half + 1) * 512],
                    ksrc[krows, kt * 128 : (kt + 1) * 128],
                    qsrc[
                        qrows,
                        qh * 1024 + half * 512 : qh * 1024 + (half + 1) * 512,
                    ],
                    start=True,
                    stop=True,
                )
            u = u_pool.tile([128, 1024], F32R, tag="u", name=f"u{name}")
            nc.scalar.activation(u[:], sc[:], AF.Exp)
            for sub in range(2):
                nc.tensor.matmul(
                    pv_tiles[sub],
                    v195[:, kt, 65 * h : 65 * h + 65],
                    u[:, sub * 512 : (sub + 1) * 512],
                    start=(kt == 0),
                    stop=(kt == TOKT - 1),
                )

        def normalize(h, qh, pv_tiles):
            for sub in range(2):
                pvt = pv_tiles[sub]
                qcol = slice(qh * 1024 + sub * 512, qh * 1024 + (sub + 1) * 512)
                nm = f"{h}_{qh}_{sub}"
                rc = rc_pool.tile([65, 512], F32R, tag="rc", name=f"rc{nm}")
                nc.vector.reciprocal(rc[64:65, :], pvt[64:65, :])
                bc = sc_pool.tile([64, 512], F32, tag="sc", name=f"bc{nm}")
                nc.tensor.matmul(
                    bc[:], ones_sb[64:65, :], rc[64:65, :], start=True, stop=True
                )
                bcs = rc_pool.tile([64, 512], F32, tag="bcs", name=f"bcs{nm}")
                nc.vector.tensor_copy(bcs[:], bc[:])
                if h == 0:
                    dest = outT01[0:64, qcol]
                elif h == 1:
                    dest = oT1tmp[:, qcol]
                else:
                    dest = outT2[:, qcol]
                nc.vector.tensor_mul(dest, pvt[0:64, :], bcs[:])

        def outproj(m):
            # output projection for token tiles 4m..4m+3; op psum borrows
            # pv-pool slots so the first half overlaps the second qh's attention
            ob = osb_pool.tile([128, D], F32, tag="osb", name=f"ob{m}")
            for j in range(4):
                n = m * 4 + j
                ts = slice(n * 128, (n + 1) * 128)
                if j > 0:
                    ob = osb_pool.tile([128, D], F32, tag="osb", name=f"ob{m}_{j}")
                for dc, (off, w) in enumerate(((0, 512), (512, 512), (1024, 256))):
                    op = pv_pool.tile([128, 512], F32, tag="pv", name=f"op{n}_{dc}")
                    nc.tensor.matmul(
                        op[:, 0:w],
                        outT01[:, ts],
                        wo01_sb[:, off : off + w],
                        start=True,
                        stop=False,
                    )
                    nc.tensor.matmul(
                        op[:, 0:w],
                        outT2[:, ts],
                        wo2_sb[:, off : off + w],
                        start=False,
                        stop=True,
                    )
                    if (n * 3 + dc) % 2 == 0:
                        nc.vector.tensor_copy(ob[:, off : off + w], op[:, 0:w])
                    else:
                        nc.scalar.copy(ob[:, off : off + w], op[:, 0:w])
                eng = nc.sync if n % 2 == 0 else nc.scalar
                eng.dma_start(out_r[n], ob[:])

        for qh in range(2):
            # heads 0,1 interleaved: their score matmuls occupy PE row groups
            # 0:64 / 64:128 and run concurrently
            pv01 = {
                h: [
                    pv_pool.tile([65, 512], F32, tag="pv", name=f"pv{qh}_{h}_{s_}")
                    for s_ in range(2)
                ]
                for h in range(2)
            }
            for kt in range(TOKT):
                for h in range(2):
                    score_pv(h, qh, kt, pv01[h], f"{qh}_{kt}_{h}")
            for h in range(2):
                normalize(h, qh, pv01[h])
            # h1's outT half into rows 64:128 of outT01 (partition-shift DMA)
            half = slice(qh * 1024, (qh + 1) * 1024)
            nc.sync.dma_start(outT01[64:128, half], oT1tmp[:, half])
            # head 2 alone
            pv2 = [
                pv_pool.tile([65, 512], F32, tag="pv", name=f"pv2_{qh}_{s_}")
                for s_ in range(2)
            ]
            for kt in range(TOKT):
                score_pv(2, qh, kt, pv2, f"{qh}_{kt}_2")
            normalize(2, qh, pv2)
            # project this query-half's tokens (overlaps the next qh's attention)
            outproj(2 * qh)
            outproj(2 * qh + 1)

        osb_pool.release()
        pv_pool.release()
        sc_pool.release()
        rc_pool.release()
        u_pool.release()

    nc.compile()
    return nc


def _get_nc():
    global _CACHED_NC
    if _CACHED_NC is None:
        _CACHED_NC = _build_nc()
    return _CACHED_NC


def _fold_cape(W, P):
    """W @ blockdiag(P) for 4x4 P repeated along channels: exact CAPE fold."""
    d = W.shape[1]
    W4 = W.reshape(W.shape[0], d // 4, 4)
    return np.einsum("cik,kj->cij", W4, P, optimize=True).reshape(W.shape[0], d)


def _prep_in_maps(hidden_states, p_out, p_out_inv, Wq, Wk, Wv, Wo):
    scale = HD ** -0.5
    hs2 = np.ascontiguousarray(hidden_states.reshape(S, D), dtype=np.float32)

    FEAT = PAD_HEADS * HD  # 1536
    Wq_eff = np.zeros((2, D, FEAT), np.float32)
    Wk_eff = np.zeros((2, D, FEAT), np.float32)
    for t in range(2):
        Wq_eff[t, :, :D] = _fold_cape(Wq, p_out_inv[0, t]) * scale
        Wk_eff[t, :, :D] = _fold_cape(Wk, p_out[0, t])
    Wv_pad = np.zeros((D, FEAT), np.float32)
    Wv_pad[:, :D] = Wv
    Wo_pad = np.zeros((FEAT, D), np.float32)
    Wo_pad[:D, :] = Wo

    def klayout(W, cols):
        # [1280, cols] -> [128, KT*cols] with ktile-major free dim
        return np.ascontiguousarray(
            W.reshape(KT, 128, cols).transpose(1, 0, 2).reshape(128, KT * cols)
        )

    ident = np.eye(128, dtype=np.float32)
    ones = np.ones((128, 64), np.float32)
    in_maps = []
    for c in range(N_CORES):
        A = c * HPC * HD
        blocks = []
        for t in range(2):
            blocks.append(klayout(Wq_eff[t][:, A : A + 128], 128))
            blocks.append(klayout(Wk_eff[t][:, A : A + 128], 128))
            blocks.append(
                klayout(
                    np.concatenate(
                        [
                            Wq_eff[t][:, A + 128 : A + 192],
                            Wk_eff[t][:, A + 128 : A + 192],
                        ],
                        axis=1,
                    ),
                    128,
                )
            )
        wv_l = klayout(
            np.concatenate(
                [Wv_pad[:, A : A + 192], np.zeros((D, 64), np.float32)], axis=1
            ),
            256,
        )
        wpack = np.ascontiguousarray(
            np.concatenate(blocks + [wv_l, ident, ones], axis=1)
        )
        assert wpack.shape == (128, WPACK_W)
        wopack = np.ascontiguousarray(
            np.concatenate(
                [
                    Wo_pad[A : A + 128, :],
                    np.concatenate(
                        [
                            Wo_pad[A + 128 : A + 192, :],
                            np.zeros((64, D), np.float32),
                        ],
                        axis=0,
                    ),
                ],
                axis=1,
            )
        )
        in_maps.append({"hs": hs2, "wpack": wpack, "wopack": wopack})
    return in_maps


def kernel(hidden_states, p_out, p_out_inv, Wq, Wk, Wv, Wo, bo):
    hidden_states = np.asarray(hidden_states, dtype=np.float32)
    in_maps = _prep_in_maps(
        hidden_states,
        np.asarray(p_out, np.float32),
        np.asarray(p_out_inv, np.float32),
        np.asarray(Wq, np.float32),
        np.asarray(Wk, np.float32),
        np.asarray(Wv, np.float32),
        np.asarray(Wo, np.float32),
    )
    nc = _get_nc()
    res = run_bass_kernel_spmd(nc, in_maps, core_ids=list(range(N_CORES)))
    acc = np.zeros((S, D), np.float32)
    for c in range(N_CORES):
        acc += res.results[c]["out"]
    acc += np.asarray(bo, np.float32)[None, :]
    out = acc.reshape(2, L, D) + hidden_states
    return out



# revision 7
# speedup vs baseline: 1.1814x; 1.1814x over previous
"""TRN2 Bass kernel for nn_AttentionStoreProcessor (dense transformer attention).

Full (unsharded) inputs in, full output out. Internally:
  - 20 heads = 8 cores x (2 full heads + 1 half-query head). SPMD-uniform
    program: odd cores see the token axis rolled by 1024 (attention is
    permutation-equivariant over keys; CAPE frame weights are swapped on the
    host so each local chunk uses its real frame).
  - CAPE rotation + softmax scale folded into Wq/Wk on host.
  - hs arrives bf16; hsT built by XBAR DMA-transpose (no PE transposes).
  - Projections in bf16; q/k re-quantized to fp8e4 and DMA-rearranged into
    the [32, 2, S] DoubleRow layout; scores run as fp8 DoubleRow matmuls
    (2 contraction tiles/instr at 0.5 cyc/row).
  - exp split between ACT (true exp -> bf16) and DVE (Schraudolph: one
    tensor_scalar to int16, bitcast to bf16).
  - PV in bf16 with a ones-column in v for the softmax sums; normalize via
    DVE reciprocal + PE broadcast matmul; per-head outputs written as fp8.
  - Output projection as fp8 DoubleRow (contraction 2x128 in one instr);
    out DMA'd as bf16; host adds bias + residual and un-rolls odd cores.
"""
import numpy as np
import ml_dtypes
from contextlib import ExitStack

import concourse.bacc as bacc
import concourse.mybir as mybir
import concourse.tile as tile
from concourse.bass_utils import run_bass_kernel_spmd

F32 = mybir.dt.float32
F32R = mybir.dt.float32r
BF16 = mybir.dt.bfloat16
FP8 = mybir.dt.float8e4
I16 = mybir.dt.int16
AF = mybir.ActivationFunctionType
ALU = mybir.AluOpType
MPM = mybir.MatmulPerfMode

HEADS = 20
N_CORES = 8
S = 2048
D = 1280
HD = 64
L = 1024
KT = D // 128  # 10 contraction tiles for projections
TOKT = S // 128  # 16 token tiles

A_SCH = 128.0 / np.log(2.0)
B_SCH = 127.0 * 128.0 - 5.5

_CACHED_NC = None


def _build_nc():
    nc = bacc.Bacc("TRN2", debug=False, num_devices=N_CORES)

    hs = nc.dram_tensor("hs", [S, D], BF16, kind="ExternalInput").ap()
    wg = nc.dram_tensor("wg", [128, 6 * KT * 128], BF16, kind="ExternalInput").ap()
    wv = nc.dram_tensor("wv", [128, KT * 192], BF16, kind="ExternalInput").ap()
    wo8d = nc.dram_tensor("wo8", [128, 2, D], FP8, kind="ExternalInput").ap()
    out = nc.dram_tensor("out", [S, D], BF16, kind="ExternalOutput").ap()
    out_r = out.rearrange("(n p) d -> n p d", p=128)

    with (
        tile.TileContext(nc) as tc,
        ExitStack() as ctx,
        nc.allow_low_precision(reason="bf16/fp8 used deliberately; tol is 2e-2"),
    ):
        persist = ctx.enter_context(tc.tile_pool(name="persist", bufs=1))
        v195 = persist.tile([128, TOKT, 195], BF16, tag="v195")
        outT = persist.tile([128, 2, S], FP8, tag="outT")
        oT1 = persist.tile([64, S], FP8, tag="oT1")
        ones = persist.tile([65, 64], F32, tag="ones")
        wo8 = persist.tile([128, 2, D], FP8, tag="wo8")
        qd = [
            persist.tile([32, 2, S], FP8, tag="qd0", name="qd0"),
            persist.tile([32, 2, S], FP8, tag="qd1", name="qd1"),
            persist.tile([32, 2, L], FP8, tag="qd2", name="qd2"),
        ]
        kd = [
            persist.tile([32, 2, S], FP8, tag="kd0", name="kd0"),
            persist.tile([32, 2, S], FP8, tag="kd1", name="kd1"),
            persist.tile([32, 2, S], FP8, tag="kd2", name="kd2"),
        ]

        # constants / zero fills (DVE is idle at startup)
        nc.vector.memset(ones[64:65, :], 1.0)
        nc.vector.memset(outT[:, 1, :], 0.0)
        for h in range(3):
            nc.vector.memset(v195[:, :, 65 * h + 64], 1.0)

        # ---- phase 1: load + transpose + projections ----
        s1 = tc.alloc_tile_pool(name="s1", bufs=1)
        wg_sb = s1.tile([128, 6 * KT * 128], BF16, tag="wg")
        wv_sb = s1.tile([128, KT * 192], BF16, tag="wv")
        hsT = [s1.tile([128, S], BF16, tag=f"hsT{k}", name=f"hsT{k}") for k in range(KT)]
        qk8 = [s1.tile([128, S], FP8, tag=f"qk8{g}", name=f"qk8{g}") for g in range(3)]

        # weights on the ACT hwdge queue; transposes on the SP queue
        half_w = 3 * KT * 128
        nc.scalar.dma_start(wg_sb[:, 0:half_w], wg[:, 0:half_w])
        for ch in range(4):
            for kt in range(KT):
                nc.sync.dma_start_transpose(
                    hsT[kt][:, ch * 512 : (ch + 1) * 512],
                    hs[ch * 512 : (ch + 1) * 512, kt * 128 : (kt + 1) * 128],
                )
            if ch == 0:
                nc.scalar.dma_start(wg_sb[:, half_w:], wg[:, half_w:])
            elif ch == 1:
                nc.scalar.dma_start(wv_sb[:], wv)
            elif ch == 2:
                nc.scalar.dma_start(wo8[:], wo8d)

        pp_ps = tc.alloc_tile_pool(name="pp", bufs=2, space="PSUM")
        vp_ps = tc.alloc_tile_pool(name="vp", bufs=2, space="PSUM")

        for ch in range(4):
            t = ch // 2
            qs = slice(ch * 512, (ch + 1) * 512)
            for g in range(3):
                pp = pp_ps.tile([128, 512], F32, tag="pp", name=f"pp{ch}_{g}")
                base = ((t * 3 + g) * KT) * 128
                for kt in range(KT):
                    nc.tensor.matmul(
                        pp[:],
                        wg_sb[:, base + kt * 128 : base + (kt + 1) * 128],
                        hsT[kt][:, qs],
                        start=(kt == 0),
                        stop=(kt == KT - 1),
                    )
                nc.vector.tensor_copy(qk8[g][:, qs], pp[:])
            for j in range(4):
                n = ch * 4 + j
                ts = slice(n * 128, (n + 1) * 128)
                vp = vp_ps.tile([128, 192], F32, tag="vp", name=f"vp{n}")
                for kt in range(KT):
                    nc.tensor.matmul(
                        vp[:],
                        hsT[kt][:, ts],
                        wv_sb[:, kt * 192 : (kt + 1) * 192],
                        start=(kt == 0),
                        stop=(kt == KT - 1),
                    )
                src = vp[:].rearrange("p (h x) -> p h x", h=3)
                dst = v195[:, n, :].rearrange("p (h x) -> p h x", h=3)[:, :, 0:64]
                if j % 2 == 0:
                    nc.scalar.copy(dst, src)
                else:
                    nc.vector.tensor_copy(dst, src)

        # q/k fp8 rearrange into DoubleRow layout [32, 2, *]
        for i in range(2):
            r0, r1 = 32 * i, 32 * (i + 1)
            nc.sync.dma_start(qd[0][:, i, :], qk8[0][r0:r1, :])
            nc.scalar.dma_start(qd[1][:, i, :], qk8[0][64 + r0 : 64 + r1, :])
            nc.sync.dma_start(kd[0][:, i, :], qk8[1][r0:r1, :])
            nc.scalar.dma_start(kd[1][:, i, :], qk8[1][64 + r0 : 64 + r1, :])
            nc.sync.dma_start(qd[2][:, i, :], qk8[2][r0:r1, 0:L])
            nc.scalar.dma_start(kd[2][:, i, :], qk8[2][64 + r0 : 64 + r1, :])

        vp_ps.release()
        pp_ps.release()
        s1.release()

        # ---- phase 2: attention ----
        sc0_ps = tc.alloc_tile_pool(name="sc0", bufs=1, space="PSUM")
        sc1_ps = tc.alloc_tile_pool(name="sc1", bufs=1, space="PSUM")
        pv0_ps = tc.alloc_tile_pool(name="pv0", bufs=1, space="PSUM")
        pv1_ps = tc.alloc_tile_pool(name="pv1", bufs=1, space="PSUM")
        aux_ps = tc.alloc_tile_pool(name="aux", bufs=1, space="PSUM")
        ua_pool = tc.alloc_tile_pool(name="ua", bufs=3)
        ui_pool = tc.alloc_tile_pool(name="ui", bufs=2)
        us_pool = tc.alloc_tile_pool(name="us", bufs=4)
        uah_pool = tc.alloc_tile_pool(name="uah", bufs=2)
        rc_pool = tc.alloc_tile_pool(name="rc", bufs=2)
        bcs_pool = tc.alloc_tile_pool(name="bcs", bufs=2)

        def dr_score(dst, h, kt, qoff, w):
            nc.tensor.matmul(
                dst,
                kd[h][:, :, kt * 128 : (kt + 1) * 128],
                qd[h][:, :, qoff : qoff + w],
                start=True,
                stop=True,
                perf_mode=MPM.DoubleRow,
            )

        def pv_mm(pv_t, col, h, kt, u_rhs):
            nc.tensor.matmul(
                pv_t[:, col : col + 512],
                v195[:, kt, 65 * h : 65 * h + 65],
                u_rhs,
                start=(kt == 0),
                stop=(kt == TOKT - 1),
            )

        def normalize(h, pv_t, qcol, nm):
            rc = rc_pool.tile([65, 1024], F32R, tag="rc", name=f"rc{nm}")
            nc.vector.reciprocal(rc[64:65, :], pv_t[64:65, :])
            bcs = bcs_pool.tile([64, 1024], F32, tag="bcs", name=f"bcs{nm}")
            for s_ in range(2):
                bc = aux_ps.tile([128, 512], F32, tag="aux", name=f"bc{nm}_{s_}")
                nc.tensor.matmul(
                    bc[0:64, :],
                    ones[64:65, :].bitcast(F32R),
                    rc[64:65, s_ * 512 : (s_ + 1) * 512],
                    start=True,
                    stop=True,
                )
                nc.scalar.copy(bcs[:, s_ * 512 : (s_ + 1) * 512], bc[0:64, :])
            if h == 0:
                dest = outT[0:64, 0, qcol]
            elif h == 1:
                dest = oT1[:, qcol]
            else:
                dest = outT[0:64, 1, qcol]
            nc.vector.tensor_mul(dest, pv_t[0:64, :], bcs[:])

        # h2 half-head pass over local queries 0:1024
        pv2 = pv0_ps.tile([65, 1024], F32, tag="pv0", name="pv2")
        for kt in range(TOKT):
            sc = sc0_ps.tile([128, 1024], F32, tag="sc0", name=f"sch2_{kt}")
            dr_score(sc[:, 0:512], 2, kt, 0, 512)
            dr_score(sc[:, 512:1024], 2, kt, 512, 512)
            if kt % 2 == 0:
                u = ua_pool.tile([128, 1024], BF16, tag="ua", name=f"uh2_{kt}")
                nc.scalar.activation(u[:], sc[:], AF.Exp)
                urhs = u
            else:
                u = ui_pool.tile([128, 1024], I16, tag="ui", name=f"uh2_{kt}")
                nc.vector.tensor_scalar(u[:], sc[:], A_SCH, B_SCH, ALU.mult, ALU.add)
                urhs = u[:].bitcast(BF16)
            pv_mm(pv2, 0, 2, kt, urhs[:, 0:512] if kt % 2 == 0 else urhs[:, 0:512])
            pv_mm(pv2, 512, 2, kt, urhs[:, 512:1024])
        normalize(2, pv2, slice(0, L), "h2")

        # main passes: heads h0 (big-tile ACT exp) + h1 (512-sub DVE exp)
        for qh in range(2):
            q0 = qh * 1024
            pva = pv0_ps.tile([65, 1024], F32, tag="pv0", name=f"pva{qh}")
            pvb = pv1_ps.tile([65, 1024], F32, tag="pv1", name=f"pvb{qh}")
            for kt in range(TOKT):
                sca = sc0_ps.tile([128, 1024], F32, tag="sc0", name=f"sca{qh}_{kt}")
                dr_score(sca[:, 0:512], 0, kt, q0, 512)
                dr_score(sca[:, 512:1024], 0, kt, q0 + 512, 512)
                ua = ua_pool.tile([128, 1024], BF16, tag="ua", name=f"ua{qh}_{kt}")
                nc.scalar.activation(ua[:], sca[:], AF.Exp)
                for s_ in range(2):
                    scb = sc1_ps.tile([128, 512], F32, tag="sc1", name=f"scb{qh}_{kt}_{s_}")
                    dr_score(scb[:], 1, kt, q0 + s_ * 512, 512)
                    if kt % 4 == 3 and s_ == 1:
                        ub = uah_pool.tile([128, 512], BF16, tag="uah", name=f"ub{qh}_{kt}_{s_}")
                        nc.scalar.activation(ub[:], scb[:], AF.Exp)
                        ubr = ub[:]
                    else:
                        ub = us_pool.tile([128, 512], I16, tag="us", name=f"ub{qh}_{kt}_{s_}")
                        nc.vector.tensor_scalar(ub[:], scb[:], A_SCH, B_SCH, ALU.mult, ALU.add)
                        ubr = ub[:].bitcast(BF16)
                    pv_mm(pvb, s_ * 512, 1, kt, ubr)
                pv_mm(pva, 0, 0, kt, ua[:, 0:512])
                pv_mm(pva, 512, 0, kt, ua[:, 512:1024])
            qcol = slice(q0, q0 + 1024)
            normalize(0, pva, qcol, f"a{qh}")
            normalize(1, pvb, qcol, f"b{qh}")
            nc.sync.dma_start(outT[64:128, 0, qcol], oT1[:, qcol])

        bcs_pool.release()
        rc_pool.release()
        uah_pool.release()
        us_pool.release()
        ui_pool.release()
        ua_pool.release()
        aux_ps.release()
        pv1_ps.release()
        pv0_ps.release()
        sc1_ps.release()
        sc0_ps.release()

        # ---- output projection (fp8 DoubleRow) ----
        op_ps = tc.alloc_tile_pool(name="op", bufs=3, space="PSUM")
        ob_pool = tc.alloc_tile_pool(name="ob", bufs=4)
        for n in range(TOKT):
            ts = slice(n * 128, (n + 1) * 128)
            ob = ob_pool.tile([128, D], BF16, tag="ob", name=f"ob{n}")
            for dc, (off, w) in enumerate(((0, 512), (512, 512), (1024, 256))):
                op = op_ps.tile([128, 512], F32, tag="op", name=f"op{n}_{dc}")
                nc.tensor.matmul(
                    op[:, 0:w],
                    outT[:, :, ts],
                    wo8[:, :, off : off + w],
                    start=True,
                    stop=True,
                    perf_mode=MPM.DoubleRow,
                )
                if (n * 3 + dc) % 2 == 0:
                    nc.vector.tensor_copy(ob[:, off : off + w], op[:, 0:w])
                else:
                    nc.scalar.copy(ob[:, off : off + w], op[:, 0:w])
            eng = nc.sync if n % 2 == 0 else nc.scalar
            eng.dma_start(out_r[n], ob[:])
        ob_pool.release()
        op_ps.release()

    nc.compile()
    return nc


def _get_nc():
    global _CACHED_NC
    if _CACHED_NC is None:
        _CACHED_NC = _build_nc()
    return _CACHED_NC


def _fold_cape(W, P):
    """W @ blockdiag(P) for 4x4 P repeated along channels: exact CAPE fold."""
    d = W.shape[1]
    W4 = W.reshape(W.shape[0], d // 4, 4)
    return np.einsum("cik,kj->cij", W4, P, optimize=True).reshape(W.shape[0], d)


def _klayout(W):
    """[1280, C] -> [128, KT*C] with ktile-major free dim."""
    C = W.shape[1]
    return np.ascontiguousarray(
        W.reshape(KT, 128, C).transpose(1, 0, 2).reshape(128, KT * C)
    )


def _prep_in_maps(hidden_states, p_out, p_out_inv, Wq, Wk, Wv, Wo):
    scale = HD ** -0.5
    hs2 = np.ascontiguousarray(hidden_states.reshape(S, D), dtype=np.float32)
    hsb = hs2.astype(ml_dtypes.bfloat16)

    Wq_eff = [(_fold_cape(Wq, p_out_inv[0, t]) * scale).astype(np.float32) for t in range(2)]
    Wk_eff = [_fold_cape(Wk, p_out[0, t]).astype(np.float32) for t in range(2)]

    def cols(W, h):
        return W[:, h * HD : (h + 1) * HD]

    in_maps = []
    for c in range(N_CORES):
        p = c // 2
        if c % 2 == 0:
            fa, fb = 5 * p, 5 * p + 1
        else:
            fa, fb = 5 * p + 2, 5 * p + 3
        fc = 5 * p + 4
        roll = (c % 2) * L
        hs_c = np.roll(hsb, -roll, axis=0) if roll else hsb
        frames = (0, 1) if c % 2 == 0 else (1, 0)
        blocks = []
        for t_real in frames:
            G0 = np.concatenate([cols(Wq_eff[t_real], fa), cols(Wq_eff[t_real], fb)], 1)
            G1 = np.concatenate([cols(Wk_eff[t_real], fa), cols(Wk_eff[t_real], fb)], 1)
            G2 = np.concatenate([cols(Wq_eff[t_real], fc), cols(Wk_eff[t_real], fc)], 1)
            blocks += [_klayout(G0), _klayout(G1), _klayout(G2)]
        wg_c = np.concatenate(blocks, axis=1).astype(ml_dtypes.bfloat16)
        wv_c = _klayout(
            np.concatenate([cols(Wv, fa), cols(Wv, fb), cols(Wv, fc)], 1)
        ).astype(ml_dtypes.bfloat16)
        tile0 = np.concatenate([Wo[fa * HD : (fa + 1) * HD], Wo[fb * HD : (fb + 1) * HD]], 0)
        tile1 = np.concatenate([Wo[fc * HD : (fc + 1) * HD], np.zeros((64, D), np.float32)], 0)
        wo8_c = np.ascontiguousarray(
            np.stack([tile0, tile1], axis=1)
        ).astype(ml_dtypes.float8_e4m3fn)
        in_maps.append(
            {
                "hs": np.ascontiguousarray(hs_c),
                "wg": wg_c,
                "wv": wv_c,
                "wo8": wo8_c,
            }
        )
    return in_maps


def kernel(hidden_states, p_out, p_out_inv, Wq, Wk, Wv, Wo, bo):
    hidden_states = np.asarray(hidden_states, dtype=np.float32)
    in_maps = _prep_in_maps(
        hidden_states,
        np.asarray(p_out, np.float32),
        np.asarray(p_out_inv, np.float32),
        np.asarray(Wq, np.float32),
        np.asarray(Wk, np.float32),
        np.asarray(Wv, np.float32),
        np.asarray(Wo, np.float32),
    )
    nc = _get_nc()
    res = run_bass_kernel_spmd(nc, in_maps, core_ids=list(range(N_CORES)))
    acc = np.zeros((S, D), np.float32)
    for c in range(N_CORES):
        o = res.results[c]["out"].astype(np.float32)
        roll = (c % 2) * L
        acc += np.roll(o, roll, axis=0) if roll else o
    acc += np.asarray(bo, np.float32)[None, :]
    out = acc.reshape(2, L, D) + hidden_states
    return out
